# revision 20
# baseline (speedup 1.0000x reference)
"""Multi-head distance (attention) layer on 8 TRN2 NeuronCores.

Sharding: data-parallel over batch. B=8 -> one batch element per core.
Each core computes a full multi-head self-attention for its [L=1024, D=256]
slice with H=8 heads of dim 64. No collectives needed.

The ScalarE exp stream is the pacing engine (~55us of pure streaming at
1 col/cycle @1.2GHz is unavoidable for the 8M softmax elements per core),
so the kernel keeps ScalarE exp-only and minimizes time outside the stream:
  - x is shipped pre-transposed (xT) and pre-pos-encoded (qkT = xT + peT),
    host-side layout prep, so there is no on-device transpose stage, no
    pos-enc add, and every input DMA descriptor line is >= 1KB.
  - input DMAs are spread over the SP/ACT/Pool queues with the slices the
    first K/Q projection actually reads (l2=0 halves, j=0 weight columns)
    ordered first, so the first S tile fills ~9us earlier than a whole-
    tensor transfer order would allow.
  - a chain of dummy matmuls on a zeroed scratch tile ramps the PE p-state
    (0.65 -> 2.4GHz needs ~3us of continuous execution) while the input
    DMAs are in flight, so the real fills run at full clock immediately.
  - all PSUM drains run on DVE; zero-fills run on DVE (startup-critical
    ones) or GpSimd (the rest); ScalarE issues nothing but exp.
Per-core algorithm (all matmul operands fp16; fp16 has fp32-grade mantissa
for this problem's value ranges):
  qT   = Wq.T @ qkT + bq        per 128-row block, bias added in the drain
  kTz  = Wk.T @ qkT             per-head tiles, other head's rows zeroed
                                (so S contracts K=128; K=64 runs half-rate)
  v    = xT.T @ Wv              [m, j] blocks, 65th column of ones appended
  per head h:
    sT[m,l] = sum_d kTz[d,m] qT[d,l]     matmul, K=128 (zero-masked)
    eT      = exp(0.125 * sT)            ScalarE, PSUM->SBUF, fp16, one call
                                         per [128, 1024] PSUM tile
    O[l,d]+Z = eT.T @ [v_h | 1]          matmul, 4 column-groups per PSUM bank
    out_h   = O * (1/Z)                  DVE reciprocal + broadcast multiply
The S/exp stream is software-pipelined (S(h+1) halves interleave with O(h)
quads) so the in-order PE queue never blocks the ScalarE exp stream.
Bias handling: bq added during the qT drain; bk only shifts score rows by a
constant (softmax-invariant) so it is dropped; bv shifts the output by
exactly repeat(bv, 64) because softmax rows sum to 1, added on the host.
Output is staged fp16 (halves the 2MB out DMA) and written back in two
whole-row DMAs (1KB contiguous lines) once the last head lands; the host
upcasts and adds bv.
"""

import numpy as np
import ml_dtypes

import concourse.bass as bass
import concourse.mybir as mybir
import concourse.tile as tile
from concourse import bacc
from concourse.bass_utils import run_bass_kernel_spmd

B, L, D = 8, 1024, 256
H, HD = 8, 64
J = H * HD  # 512
TEMPERATURE = 10000.0

f32 = mybir.dt.float32
bf16 = mybir.dt.float16  # fp16: same PE rate as bf16, 8x the mantissa

_CACHE = {}
LAST_RESULT = None  # BassKernelResults of the most recent run (for profiling)
TRACE = False

STILE = 1024  # S-chunk PSUM/exp tile width (2 chunks of 512 = 2 banks)


def _emit(tc, aps):
    nc = tc.nc
    Exp = mybir.ActivationFunctionType.Exp
    xt, qkt, wq, wk, wv, bqc, out = (
        aps["xt"], aps["qkt"], aps["wq"], aps["wk"], aps["wv"], aps["bqc"],
        aps["out"],
    )

    xtr = xt.rearrange("(t p) l -> t p l", p=128)        # [2, 128, 1024]
    qktr = qkt.rearrange("(t p) l -> t p l", p=128)      # [2, 128, 1024]
    wqr = wq.rearrange("(t p) j -> t p j", p=128)        # [2, 128, 512]
    wkr = wk.rearrange("(t p) j -> t p j", p=128)
    wvr = wv.rearrange("(t p) j -> t p j", p=128)
    outr = out.rearrange("(n p) j -> p n j", p=128)      # [128, 8, 512]

    import contextlib
    ctx = contextlib.ExitStack()
    persist = ctx.enter_context(tc.tile_pool(name="persist", bufs=1))
    epool = ctx.enter_context(tc.tile_pool(name="epool", bufs=20))
    rpool = ctx.enter_context(tc.tile_pool(name="rpool", bufs=4))
    s_ps = ctx.enter_context(tc.tile_pool(name="sps", bufs=3, space="PSUM"))
    o_ps = ctx.enter_context(tc.tile_pool(name="ops", bufs=2, space="PSUM"))

    # --- input DMAs over 3 queues; the slices the first projections read
    # (qkT l2=0 halves, j=0 weight columns, bq) lead their queues ---
    qkT = [persist.tile([128, 1024], bf16, name=f"qkT{t}") for t in range(2)]
    xT = [persist.tile([128, 1024], bf16, name=f"xT{t}") for t in range(2)]
    w_sb = {
        wname: [persist.tile([128, 512], bf16, name=f"{wname}_sb{t}")
                for t in range(2)]
        for wname in ("wq", "wk", "wv")
    }
    bq_sb = persist.tile([128, 4], f32, name="bq_sb")

    nc.sync.dma_start(out=bq_sb[:], in_=bqc[:, :])
    nc.sync.dma_start(out=qkT[0][:, 0:512], in_=qktr[0][:, 0:512])
    nc.sync.dma_start(out=w_sb["wk"][0][:, 0:128], in_=wkr[0][:, 0:128])
    nc.sync.dma_start(out=qkT[0][:, 512:1024], in_=qktr[0][:, 512:1024])
    nc.sync.dma_start(out=w_sb["wk"][0][:, 128:512], in_=wkr[0][:, 128:512])
    nc.sync.dma_start(out=xT[0][:], in_=xtr[0])

    nc.scalar.dma_start(out=qkT[1][:, 0:512], in_=qktr[1][:, 0:512])
    nc.scalar.dma_start(out=w_sb["wk"][1][:, 0:128], in_=wkr[1][:, 0:128])

    nc.gpsimd.dma_start(out=w_sb["wq"][0][:, 0:128], in_=wqr[0][:, 0:128])
    nc.gpsimd.dma_start(out=w_sb["wq"][1][:, 0:128], in_=wqr[1][:, 0:128])

    # --- startup-critical zero-fills on the otherwise-idle DVE queue ---
    scratch = persist.tile([128, 512], bf16, name="scratch")
    nc.vector.memset(scratch[:], 0.0)
    kTz = [persist.tile([128, 1024], bf16, name=f"kTz{h}") for h in range(8)]
    for h in range(2):
        nc.vector.memset(kTz[h][:], 0.0)
    sc_in = persist.tile([128, 8], f32, name="sc_in")
    sc_out = persist.tile([128, 8], f32, name="sc_out")
    nc.vector.memset(sc_in[:], 0.0)

    # ACT exp-table preload, then the rest of the scalar queue's input DMAs
    nc.scalar.activation(sc_out[:], sc_in[:], Exp)
    nc.scalar.dma_start(out=qkT[1][:, 512:1024], in_=qktr[1][:, 512:1024])
    nc.scalar.dma_start(out=w_sb["wk"][1][:, 128:512], in_=wkr[1][:, 128:512])

    # remaining inputs + non-critical zero-fills behind the gpsimd queue
    nc.gpsimd.dma_start(out=w_sb["wq"][0][:, 128:512], in_=wqr[0][:, 128:512])
    nc.gpsimd.dma_start(out=w_sb["wq"][1][:, 128:512], in_=wqr[1][:, 128:512])
    nc.gpsimd.dma_start(out=w_sb["wv"][0][:], in_=wvr[0])
    nc.gpsimd.dma_start(out=w_sb["wv"][1][:], in_=wvr[1])
    nc.gpsimd.dma_start(out=xT[1][:], in_=xtr[1])
    for h in range(2, 8):
        nc.gpsimd.memset(kTz[h][:], 0.0)

    # --- PE p-state warmup: the tensor engine needs ~3us of continuous
    # execution to reach 2.4GHz (it starts at 0.65GHz); dummy matmuls on the
    # zeroed scratch tile, dependent only on the scratch memset, ramp it
    # while the input DMAs are in flight so the real projection and S fills
    # run at full clock from their first instruction. ---
    warm = s_ps.tile([128, 512], f32, tag="s", name="warm")
    for _ in range(7):
        nc.tensor.matmul(warm[:, 0:512], lhsT=scratch[:, 0:128],
                         rhs=scratch[:], start=True, stop=True)

    # --- QKV projections (PSUM fills share the "s" tag slots) ---
    qT = [persist.tile([128, 1024], bf16, name=f"qT{j}") for j in range(4)]
    v_sb = [persist.tile([128, 8, 65], bf16, name=f"v_sb{m}") for m in range(8)]
    for m in range(8):
        nc.gpsimd.memset(v_sb[m][:, :, 64:65], 1.0)

    # projection fills allocate from the o_ps pool (1-bank tiles), NOT the
    # s_ps rotation: s_ps slots recycle only as fast as the exp stream
    # drains them, so a piece placed there stalls the PE (and with it the
    # next S fill) behind an in-flight exp; o_ps is idle until the first
    # O quad and its 2-buf rotation is drained by fast DVE copies instead.
    def qk_piece(j, which, l2):
        wname = "wq" if which == "q" else "wk"
        pq = o_ps.tile([128, 512], f32, tag="o", name="pq")
        for c2 in range(2):
            nc.tensor.matmul(
                pq[:, 0:512],
                lhsT=w_sb[wname][c2][:, j * 128:(j + 1) * 128],
                rhs=qkT[c2][:, l2 * 512:(l2 + 1) * 512],
                start=(c2 == 0),
                stop=(c2 == 1),
            )
        dsl = slice(l2 * 512, (l2 + 1) * 512)
        if which == "q":
            nc.vector.tensor_scalar_add(
                qT[j][:, dsl], pq[:, 0:512], bq_sb[:, j:j + 1]
            )
        else:
            nc.vector.tensor_copy(kTz[2 * j][0:64, dsl], pq[0:64, 0:512])
            nc.vector.tensor_copy(kTz[2 * j + 1][64:128, dsl], pq[64:128, 0:512])

    def v_proj(m):
        pv = o_ps.tile([128, 512], f32, tag="o", name="pv")
        for c2 in range(2):
            nc.tensor.matmul(
                pv[:, 0:512],
                lhsT=xT[c2][:, m * 128:(m + 1) * 128],
                rhs=w_sb["wv"][c2][:],
                start=(c2 == 0),
                stop=(c2 == 1),
            )
        nc.vector.tensor_copy(
            v_sb[m][:, :, 0:64], pv[:, 0:512].rearrange("p (h d) -> p h d", h=8)
        )

    # --- attention: S-chunks packed into [128, STILE] PSUM tiles; one exp
    # per tile. Software-pipelined: S(h+1) emitted before O(h). ---
    out_sb = persist.tile([128, 8, 512], bf16, name="out_sb")
    epos = {}  # (h, mc, l2) -> (e_tile, col_offset)
    state = {"tile": None, "off": 0, "chunks": []}

    def flush_exp():
        if state["tile"] is None or not state["chunks"]:
            return
        e = epool.tile([128, state["off"]], bf16, tag="e", name="e")
        nc.scalar.activation(
            e[:], state["tile"][:, 0:state["off"]], Exp, scale=float(HD) ** -0.5
        )
        for key, off in state["chunks"]:
            epos[key] = (e, off)
        state["tile"] = None
        state["off"] = 0
        state["chunks"] = []

    def s_chunk(h, mc, l2):
        if state["tile"] is None:
            state["tile"] = s_ps.tile([128, STILE], f32, tag="s", name="ps")
        off = state["off"]
        nc.tensor.matmul(
            state["tile"][:, off:off + 512],
            lhsT=kTz[h][:, mc * 128:(mc + 1) * 128],
            rhs=qT[h // 2][:, l2 * 512:(l2 + 1) * 512],
            start=True,
            stop=True,
        )
        state["chunks"].append(((h, mc, l2), off))
        state["off"] = off + 512
        if state["off"] == STILE:
            flush_exp()

    def emit_S_half(h, l2):
        for mc in range(8):
            s_chunk(h, mc, l2)

    def emit_O_quad(h, q):
        hsl = slice(h * 64, (h + 1) * 64)
        pO = o_ps.tile([128, 260], f32, tag="o", name="pO")
        for g in range(4):
            lc = 4 * q + g
            l2, sub = lc // 4, lc % 4
            for mc in range(8):
                e, off = epos[(h, mc, l2)]
                nc.tensor.matmul(
                    pO[:, 65 * g:65 * g + 65],
                    lhsT=e[:, off + sub * 128:off + (sub + 1) * 128],
                    rhs=v_sb[mc][:, h, :],
                    start=(mc == 0),
                    stop=(mc == 7),
                )
        pOr = pO.rearrange("p (g c) -> p g c", g=4)      # [128, 4, 65]
        if h == 7 and q == 1:
            # tail quad: drain + write back per 2-group half so the first
            # DMA overlaps the second half's drain instead of waiting for
            # the whole quad.
            for half, eng in ((0, nc.scalar), (1, nc.sync)):
                gsl = slice(2 * half, 2 * half + 2)
                nsl = slice(4 + 2 * half, 6 + 2 * half)
                rc = rpool.tile([128, 2], f32, tag="rc", name="rc")
                nc.vector.reciprocal(rc[:], pOr[:, gsl, 64])
                rcb = bass.AP(
                    tensor=rc.tensor, offset=rc.offset,
                    ap=[rc.ap[0], rc.ap[1], [0, 64]],
                )
                nc.vector.tensor_mul(
                    out_sb[:, nsl, hsl], pOr[:, gsl, 0:64], rcb
                )
                eng.dma_start(out=outr[:, nsl, :], in_=out_sb[:, nsl, :])
            return
        rc = rpool.tile([128, 4], f32, tag="rc", name="rc")
        nc.vector.reciprocal(rc[:], pOr[:, :, 64])
        rcb = bass.AP(
            tensor=rc.tensor, offset=rc.offset,
            ap=[rc.ap[0], rc.ap[1], [0, 64]],
        )
        nc.vector.tensor_mul(
            out_sb[:, 4 * q:4 * q + 4, hsl], pOr[:, :, 0:64], rcb
        )
        if h == 7:
            # n-rows 0..4 have all 8 heads: one contiguous 1KB-per-line DMA
            # for the full j-range instead of 8 strided 128B ones.
            nc.sync.dma_start(out=outr[:, 0:4, :], in_=out_sb[:, 0:4, :])

    # schedule: (head, half) S-emissions and (head, quad) O-emissions are
    # interleaved one step apart; QKV projections dropped in just before the
    # first S-half that needs them. V only feeds O so it comes after S(0).
    # k-piece l2 indexes KEY positions: S(0,0) chunks mc>=4 read kTz columns
    # 512:1024, so qk_piece(0,"k",1) must land before them — it rides mid-
    # half (its DMA slices arrive later than the l2=0 ones), covered by the
    # exps of the first two S tiles.
    qk_piece(0, "k", 0)
    qk_piece(0, "q", 0)
    s_chunk(0, 0, 0)
    flush_exp()
    for mc in range(1, 4):
        s_chunk(0, mc, 0)
    qk_piece(0, "k", 1)
    for mc in range(4, 7):
        s_chunk(0, mc, 0)
    flush_exp()
    s_chunk(0, 7, 0)
    flush_exp()
    qk_piece(0, "q", 1)
    for m in range(4):
        v_proj(m)
    emit_S_half(0, 1)
    for m in range(4, 8):
        v_proj(m)
    # qk pieces for projection j are spread across the 4 steps of head block
    # 2j-1 so they never bunch up in front of an S-fill.
    inject = {
        (1, i): (1, w, l2) for i, (w, l2) in enumerate(
            [("q", 0), ("q", 1), ("k", 0), ("k", 1)])
    }
    inject.update({(3, i): (2, w, l2) for i, (w, l2) in enumerate(
        [("q", 0), ("q", 1), ("k", 0), ("k", 1)])})
    inject.update({(5, i): (3, w, l2) for i, (w, l2) in enumerate(
        [("q", 0), ("q", 1), ("k", 0), ("k", 1)])})
    for h in range(1, 8):
        for stepi, (kind, hh, part) in enumerate(
            [("S", h, 0), ("O", h - 1, 0), ("S", h, 1), ("O", h - 1, 1)]
        ):
            if kind == "S":
                emit_S_half(hh, part)
            else:
                emit_O_quad(hh, part)
            if (h, stepi) in inject:
                j, w, l2 = inject[(h, stepi)]
                qk_piece(j, w, l2)
    emit_O_quad(7, 0)
    emit_O_quad(7, 1)
    ctx.close()


def _build():
    if "nc" in _CACHE:
        return _CACHE["nc"]
    nc = bacc.Bacc("TRN2", target_bir_lowering=False, debug=False, num_devices=8)
    aps = {
        "xt": nc.dram_tensor("xt", [D, L], bf16, kind="ExternalInput").ap(),
        "qkt": nc.dram_tensor("qkt", [D, L], bf16, kind="ExternalInput").ap(),
        "wq": nc.dram_tensor("wq", [D, J], bf16, kind="ExternalInput").ap(),
        "wk": nc.dram_tensor("wk", [D, J], bf16, kind="ExternalInput").ap(),
        "wv": nc.dram_tensor("wv", [D, J], bf16, kind="ExternalInput").ap(),
        "bqc": nc.dram_tensor("bqc", [128, 4], f32, kind="ExternalInput").ap(),
        "out": nc.dram_tensor("out", [L, J], bf16, kind="ExternalOutput").ap(),
    }
    with tile.TileContext(nc) as tc:
        _emit(tc, aps)
    nc.compile()
    _CACHE["nc"] = nc
    return nc


def _pe_T():
    embed = np.arange(L, dtype=np.float32)
    dim_t = np.arange(D, dtype=np.float32)
    dim_t = (np.float32(TEMPERATURE) ** (2.0 * np.floor(dim_t / 2.0) / np.float32(D))).astype(np.float32)
    pos = embed[:, None] / dim_t  # [L, D]
    pe = np.stack([np.sin(pos[:, 0::2]), np.cos(pos[:, 1::2])], axis=2).reshape(L, D)
    return np.ascontiguousarray(pe.T.astype(np.float32))  # [D, L]


def kernel(**inputs):
    global LAST_RESULT
    bf = np.float16
    x = np.asarray(inputs["x"], dtype=np.float32)
    wq = np.ascontiguousarray(np.asarray(inputs["Wq"], dtype=np.float32).astype(bf))
    wk = np.ascontiguousarray(np.asarray(inputs["Wk"], dtype=np.float32).astype(bf))
    wv = np.ascontiguousarray(np.asarray(inputs["Wv"], dtype=np.float32).astype(bf))
    bq = np.asarray(inputs["bq"], dtype=np.float32)
    bv = np.asarray(inputs["bv"], dtype=np.float32)

    nc = _build()
    bqc = np.ascontiguousarray(np.repeat(bq, HD).reshape(4, 128).T)  # [128, 4]
    peT = _pe_T()                                                    # [D, L]
    xT = np.swapaxes(x, 1, 2)                                        # [B, D, L]
    qkT = (xT + peT[None]).astype(bf)                                # [B, D, L]
    xT = xT.astype(bf)
    base = {"wq": wq, "wk": wk, "wv": wv, "bqc": bqc}
    in_maps = [
        {**base, "xt": np.ascontiguousarray(xT[b]),
         "qkt": np.ascontiguousarray(qkT[b])}
        for b in range(B)
    ]
    res = run_bass_kernel_spmd(
        nc, in_maps, core_ids=list(range(B)), trace=TRACE
    )
    LAST_RESULT = res
    out = np.stack([res.results[b]["out"] for b in range(B)]).astype(np.float32)
    out += np.repeat(bv, HD)[None, None, :]
    return out


# revision 22
# speedup vs baseline: 1.0125x; 1.0125x over previous
"""Multi-head distance (attention) layer on 8 TRN2 NeuronCores.

Sharding: data-parallel over batch. B=8 -> one batch element per core.
Each core computes a full multi-head self-attention for its [L=1024, D=256]
slice with H=8 heads of dim 64. No collectives needed.

The ScalarE exp stream is the pacing engine (~55us of pure streaming at
1 col/cycle @1.2GHz is unavoidable for the 8M softmax elements per core),
so the kernel keeps ScalarE exp-only and minimizes time outside the stream:
  - x is shipped pre-transposed (xT) and pre-pos-encoded (qkT = xT + peT),
    host-side layout prep, so there is no on-device transpose stage, no
    pos-enc add, and every input DMA descriptor line is >= 1KB.
  - input DMAs are spread over the SP/ACT/Pool queues with the slices the
    first K/Q projection actually reads (l2=0 halves, j=0 weight columns)
    ordered first, so the first S tile fills ~9us earlier than a whole-
    tensor transfer order would allow.
  - a chain of dummy matmuls on a zeroed scratch tile ramps the PE p-state
    (0.65 -> 2.4GHz needs ~3us of continuous execution) while the input
    DMAs are in flight, so the real fills run at full clock immediately.
  - all PSUM drains run on DVE; zero-fills run on DVE (startup-critical
    ones) or GpSimd (the rest); ScalarE issues nothing but exp.
Per-core algorithm (all matmul operands fp16; fp16 has fp32-grade mantissa
for this problem's value ranges):
  qT   = Wq.T @ qkT + bq        per 128-row block, bias added in the drain
  kTz  = Wk.T @ qkT             per-head tiles, other head's rows zeroed
                                (so S contracts K=128; K=64 runs half-rate)
  v    = xT.T @ Wv              [m, j] blocks, 65th column of ones appended
  per head h:
    sT[m,l] = sum_d kTz[d,m] qT[d,l]     matmul, K=128 (zero-masked)
    eT      = exp(0.125 * sT)            ScalarE, PSUM->SBUF, fp16, one call
                                         per [128, 1024] PSUM tile
    O[l,d]+Z = eT.T @ [v_h | 1]          matmul, 4 column-groups per PSUM bank
    out_h   = O * (1/Z)                  DVE reciprocal + broadcast multiply
The S/exp stream is software-pipelined (S(h+1) halves interleave with O(h)
quads) so the in-order PE queue never blocks the ScalarE exp stream.
Bias handling: bq added during the qT drain; bk only shifts score rows by a
constant (softmax-invariant) so it is dropped; bv shifts the output by
exactly repeat(bv, 64) because softmax rows sum to 1, added on the host.
Output is staged fp16 (halves the 2MB out DMA) and written back in two
whole-row DMAs (1KB contiguous lines) once the last head lands; the host
upcasts and adds bv.
"""

import numpy as np
import ml_dtypes

import concourse.bass as bass
import concourse.mybir as mybir
import concourse.tile as tile
from concourse import bacc
from concourse.bass_utils import run_bass_kernel_spmd

B, L, D = 8, 1024, 256
H, HD = 8, 64
J = H * HD  # 512
TEMPERATURE = 10000.0

f32 = mybir.dt.float32
bf16 = mybir.dt.float16  # fp16: same PE rate as bf16, 8x the mantissa

_CACHE = {}
LAST_RESULT = None  # BassKernelResults of the most recent run (for profiling)
TRACE = False

STILE = 1024  # S-chunk PSUM/exp tile width (2 chunks of 512 = 2 banks)


def _emit(tc, aps):
    nc = tc.nc
    Exp = mybir.ActivationFunctionType.Exp
    xt, qkt, wq, wk, wv, bqc, out = (
        aps["xt"], aps["qkt"], aps["wq"], aps["wk"], aps["wv"], aps["bqc"],
        aps["out"],
    )

    xtr = xt.rearrange("(t p) l -> t p l", p=128)        # [2, 128, 1024]
    qktr = qkt.rearrange("(t p) l -> t p l", p=128)      # [2, 128, 1024]
    wqr = wq.rearrange("(t p) j -> t p j", p=128)        # [2, 128, 512]
    wkr = wk.rearrange("(t p) j -> t p j", p=128)
    wvr = wv.rearrange("(t p) j -> t p j", p=128)
    outr = out.rearrange("(n p) j -> p n j", p=128)      # [128, 8, 512]

    import contextlib
    ctx = contextlib.ExitStack()
    persist = ctx.enter_context(tc.tile_pool(name="persist", bufs=1))
    epool = ctx.enter_context(tc.tile_pool(name="epool", bufs=20))
    rpool = ctx.enter_context(tc.tile_pool(name="rpool", bufs=4))
    s_ps = ctx.enter_context(tc.tile_pool(name="sps", bufs=3, space="PSUM"))
    o_ps = ctx.enter_context(tc.tile_pool(name="ops", bufs=2, space="PSUM"))

    # --- input DMAs over 3 queues; the slices the first projections read
    # (qkT l2=0 halves, j=0 weight columns, bq) lead their queues ---
    qkT = [persist.tile([128, 1024], bf16, name=f"qkT{t}") for t in range(2)]
    xT = [persist.tile([128, 1024], bf16, name=f"xT{t}") for t in range(2)]
    w_sb = {
        wname: [persist.tile([128, 512], bf16, name=f"{wname}_sb{t}")
                for t in range(2)]
        for wname in ("wq", "wk", "wv")
    }
    bq_sb = persist.tile([128, 4], f32, name="bq_sb")

    nc.sync.dma_start(out=bq_sb[:], in_=bqc[:, :])
    nc.sync.dma_start(out=qkT[0][:, 0:512], in_=qktr[0][:, 0:512])
    nc.sync.dma_start(out=w_sb["wk"][0][:, 0:128], in_=wkr[0][:, 0:128])
    nc.sync.dma_start(out=qkT[0][:, 512:1024], in_=qktr[0][:, 512:1024])
    nc.sync.dma_start(out=w_sb["wk"][0][:, 128:512], in_=wkr[0][:, 128:512])
    nc.sync.dma_start(out=xT[0][:], in_=xtr[0])

    nc.scalar.dma_start(out=qkT[1][:, 0:512], in_=qktr[1][:, 0:512])
    nc.scalar.dma_start(out=w_sb["wk"][1][:, 0:128], in_=wkr[1][:, 0:128])

    nc.gpsimd.dma_start(out=w_sb["wq"][0][:, 0:128], in_=wqr[0][:, 0:128])
    nc.gpsimd.dma_start(out=w_sb["wq"][1][:, 0:128], in_=wqr[1][:, 0:128])

    # --- startup-critical zero-fills on the otherwise-idle DVE queue ---
    scratch = persist.tile([128, 512], bf16, name="scratch")
    nc.vector.memset(scratch[:], 0.0)
    sc_in = persist.tile([128, 8], f32, name="sc_in")
    sc_out = persist.tile([128, 8], f32, name="sc_out")
    nc.vector.memset(sc_in[:], 0.0)
    kTz = [persist.tile([128, 1024], bf16, name=f"kTz{h}") for h in range(8)]
    for h in range(2):
        nc.vector.memset(kTz[h][:], 0.0)

    # ACT exp-table preload, then the rest of the scalar queue's input DMAs
    nc.scalar.activation(sc_out[:], sc_in[:], Exp)
    nc.scalar.dma_start(out=qkT[1][:, 512:1024], in_=qktr[1][:, 512:1024])
    nc.scalar.dma_start(out=w_sb["wk"][1][:, 128:512], in_=wkr[1][:, 128:512])

    # remaining inputs + non-critical zero-fills behind the gpsimd queue
    nc.gpsimd.dma_start(out=w_sb["wq"][0][:, 128:512], in_=wqr[0][:, 128:512])
    nc.gpsimd.dma_start(out=w_sb["wq"][1][:, 128:512], in_=wqr[1][:, 128:512])
    nc.gpsimd.dma_start(out=w_sb["wv"][0][:], in_=wvr[0])
    nc.gpsimd.dma_start(out=w_sb["wv"][1][:], in_=wvr[1])
    nc.gpsimd.dma_start(out=xT[1][:], in_=xtr[1])
    for h in range(2, 8):
        nc.gpsimd.memset(kTz[h][:], 0.0)

    # --- PE p-state warmup: the tensor engine needs ~3us of continuous
    # execution to reach 2.4GHz (it starts at 0.65GHz); dummy matmuls on the
    # zeroed scratch tile, dependent only on the scratch memset, ramp it
    # while the input DMAs are in flight so the real projection and S fills
    # run at full clock from their first instruction. ---
    warm = s_ps.tile([128, 512], f32, tag="s", name="warm")
    for _ in range(7):
        nc.tensor.matmul(warm[:, 0:512], lhsT=scratch[:, 0:128],
                         rhs=scratch[:], start=True, stop=True)

    # --- QKV projections (PSUM fills share the "s" tag slots) ---
    qT = [persist.tile([128, 1024], bf16, name=f"qT{j}") for j in range(4)]
    v_sb = [persist.tile([128, 8, 65], bf16, name=f"v_sb{m}") for m in range(8)]
    for m in range(8):
        nc.gpsimd.memset(v_sb[m][:, :, 64:65], 1.0)

    # projection fills allocate from the o_ps pool (1-bank tiles), NOT the
    # s_ps rotation: s_ps slots recycle only as fast as the exp stream
    # drains them, so a piece placed there stalls the PE (and with it the
    # next S fill) behind an in-flight exp; o_ps is idle until the first
    # O quad and its 2-buf rotation is drained by fast DVE copies instead.
    deferred = []

    def qk_piece(j, which, l2):
        wname = "wq" if which == "q" else "wk"
        pq = o_ps.tile([128, 512], f32, tag="o", name="pq")
        for c2 in range(2):
            nc.tensor.matmul(
                pq[:, 0:512],
                lhsT=w_sb[wname][c2][:, j * 128:(j + 1) * 128],
                rhs=qkT[c2][:, l2 * 512:(l2 + 1) * 512],
                start=(c2 == 0),
                stop=(c2 == 1),
            )
        dsl = slice(l2 * 512, (l2 + 1) * 512)
        if which == "q":
            nc.vector.tensor_scalar_add(
                qT[j][:, dsl], pq[:, 0:512], bq_sb[:, j:j + 1]
            )
        else:
            nc.vector.tensor_copy(kTz[2 * j][0:64, dsl], pq[0:64, 0:512])
            if j == 0:
                # head 1's half is off the S(0,*) critical path: defer it
                # behind the drains the first S fills actually wait on.
                deferred.append((kTz[1], dsl, pq))
            else:
                nc.vector.tensor_copy(
                    kTz[2 * j + 1][64:128, dsl], pq[64:128, 0:512]
                )

    def flush_deferred():
        while deferred:
            dst, dsl, pq = deferred.pop(0)
            nc.vector.tensor_copy(dst[64:128, dsl], pq[64:128, 0:512])

    def v_proj(m):
        pv = o_ps.tile([128, 512], f32, tag="o", name="pv")
        for c2 in range(2):
            nc.tensor.matmul(
                pv[:, 0:512],
                lhsT=xT[c2][:, m * 128:(m + 1) * 128],
                rhs=w_sb["wv"][c2][:],
                start=(c2 == 0),
                stop=(c2 == 1),
            )
        nc.vector.tensor_copy(
            v_sb[m][:, :, 0:64], pv[:, 0:512].rearrange("p (h d) -> p h d", h=8)
        )

    # --- attention: S-chunks packed into [128, STILE] PSUM tiles; one exp
    # per tile. Software-pipelined: S(h+1) emitted before O(h). ---
    out_sb = persist.tile([128, 8, 512], bf16, name="out_sb")
    epos = {}  # (h, mc, l2) -> (e_tile, col_offset)
    state = {"tile": None, "off": 0, "chunks": []}

    def flush_exp():
        if state["tile"] is None or not state["chunks"]:
            return
        e = epool.tile([128, state["off"]], bf16, tag="e", name="e")
        nc.scalar.activation(
            e[:], state["tile"][:, 0:state["off"]], Exp, scale=float(HD) ** -0.5
        )
        for key, off in state["chunks"]:
            epos[key] = (e, off)
        state["tile"] = None
        state["off"] = 0
        state["chunks"] = []

    def s_chunk(h, mc, l2):
        if state["tile"] is None:
            state["tile"] = s_ps.tile([128, STILE], f32, tag="s", name="ps")
        off = state["off"]
        nc.tensor.matmul(
            state["tile"][:, off:off + 512],
            lhsT=kTz[h][:, mc * 128:(mc + 1) * 128],
            rhs=qT[h // 2][:, l2 * 512:(l2 + 1) * 512],
            start=True,
            stop=True,
        )
        state["chunks"].append(((h, mc, l2), off))
        state["off"] = off + 512
        if state["off"] == STILE:
            flush_exp()

    def emit_S_half(h, l2):
        for mc in range(8):
            s_chunk(h, mc, l2)

    def emit_O_quad(h, q):
        hsl = slice(h * 64, (h + 1) * 64)
        pO = o_ps.tile([128, 260], f32, tag="o", name="pO")
        for g in range(4):
            lc = 4 * q + g
            l2, sub = lc // 4, lc % 4
            for mc in range(8):
                e, off = epos[(h, mc, l2)]
                nc.tensor.matmul(
                    pO[:, 65 * g:65 * g + 65],
                    lhsT=e[:, off + sub * 128:off + (sub + 1) * 128],
                    rhs=v_sb[mc][:, h, :],
                    start=(mc == 0),
                    stop=(mc == 7),
                )
        pOr = pO.rearrange("p (g c) -> p g c", g=4)      # [128, 4, 65]
        rc = rpool.tile([128, 4], f32, tag="rc", name="rc")
        nc.vector.reciprocal(rc[:], pOr[:, :, 64])
        rcb = bass.AP(
            tensor=rc.tensor, offset=rc.offset,
            ap=[rc.ap[0], rc.ap[1], [0, 64]],
        )
        nc.vector.tensor_mul(
            out_sb[:, 4 * q:4 * q + 4, hsl], pOr[:, :, 0:64], rcb
        )
        if h == 7:
            # n-rows 4q..4q+4 now have all 8 heads: contiguous 1KB-per-line
            # DMAs for the full j-range instead of 8 strided 128B ones. The
            # final quad's transfer is the kernel tail, so it is split
            # across two queues to halve its wall time.
            if q == 0:
                nc.sync.dma_start(out=outr[:, 0:4, :], in_=out_sb[:, 0:4, :])
            else:
                nc.scalar.dma_start(out=outr[:, 4:6, :], in_=out_sb[:, 4:6, :])
                nc.sync.dma_start(out=outr[:, 6:8, :], in_=out_sb[:, 6:8, :])

    # schedule: (head, half) S-emissions and (head, quad) O-emissions are
    # interleaved one step apart; QKV projections dropped in just before the
    # first S-half that needs them. V only feeds O so it comes after S(0).
    # k-piece l2 indexes KEY positions: S(0,0) chunks mc>=4 read kTz columns
    # 512:1024, so qk_piece(0,"k",1) must land before them — it rides mid-
    # half (its DMA slices arrive later than the l2=0 ones), covered by the
    # exps of the first two S tiles.
    qk_piece(0, "k", 0)
    qk_piece(0, "q", 0)
    flush_deferred()
    for mc in range(4):
        s_chunk(0, mc, 0)
    qk_piece(0, "k", 1)
    flush_deferred()
    for mc in range(4, 8):
        s_chunk(0, mc, 0)
    qk_piece(0, "q", 1)
    for m in range(4):
        v_proj(m)
    emit_S_half(0, 1)
    for m in range(4, 8):
        v_proj(m)
    # qk pieces for projection j are spread across the 4 steps of head block
    # 2j-1 so they never bunch up in front of an S-fill.
    inject = {
        (1, i): (1, w, l2) for i, (w, l2) in enumerate(
            [("q", 0), ("q", 1), ("k", 0), ("k", 1)])
    }
    inject.update({(3, i): (2, w, l2) for i, (w, l2) in enumerate(
        [("q", 0), ("q", 1), ("k", 0), ("k", 1)])})
    inject.update({(5, i): (3, w, l2) for i, (w, l2) in enumerate(
        [("q", 0), ("q", 1), ("k", 0), ("k", 1)])})
    for h in range(1, 8):
        for stepi, (kind, hh, part) in enumerate(
            [("S", h, 0), ("O", h - 1, 0), ("S", h, 1), ("O", h - 1, 1)]
        ):
            if kind == "S":
                emit_S_half(hh, part)
            else:
                emit_O_quad(hh, part)
            if (h, stepi) in inject:
                j, w, l2 = inject[(h, stepi)]
                qk_piece(j, w, l2)
    emit_O_quad(7, 0)
    emit_O_quad(7, 1)
    ctx.close()


def _build():
    if "nc" in _CACHE:
        return _CACHE["nc"]
    nc = bacc.Bacc("TRN2", target_bir_lowering=False, debug=False, num_devices=8)
    aps = {
        "xt": nc.dram_tensor("xt", [D, L], bf16, kind="ExternalInput").ap(),
        "qkt": nc.dram_tensor("qkt", [D, L], bf16, kind="ExternalInput").ap(),
        "wq": nc.dram_tensor("wq", [D, J], bf16, kind="ExternalInput").ap(),
        "wk": nc.dram_tensor("wk", [D, J], bf16, kind="ExternalInput").ap(),
        "wv": nc.dram_tensor("wv", [D, J], bf16, kind="ExternalInput").ap(),
        "bqc": nc.dram_tensor("bqc", [128, 4], f32, kind="ExternalInput").ap(),
        "out": nc.dram_tensor("out", [L, J], bf16, kind="ExternalOutput").ap(),
    }
    with tile.TileContext(nc) as tc:
        _emit(tc, aps)
    nc.compile()
    _CACHE["nc"] = nc
    return nc


def _pe_T():
    embed = np.arange(L, dtype=np.float32)
    dim_t = np.arange(D, dtype=np.float32)
    dim_t = (np.float32(TEMPERATURE) ** (2.0 * np.floor(dim_t / 2.0) / np.float32(D))).astype(np.float32)
    pos = embed[:, None] / dim_t  # [L, D]
    pe = np.stack([np.sin(pos[:, 0::2]), np.cos(pos[:, 1::2])], axis=2).reshape(L, D)
    return np.ascontiguousarray(pe.T.astype(np.float32))  # [D, L]


def kernel(**inputs):
    global LAST_RESULT
    bf = np.float16
    x = np.asarray(inputs["x"], dtype=np.float32)
    wq = np.ascontiguousarray(np.asarray(inputs["Wq"], dtype=np.float32).astype(bf))
    wk = np.ascontiguousarray(np.asarray(inputs["Wk"], dtype=np.float32).astype(bf))
    wv = np.ascontiguousarray(np.asarray(inputs["Wv"], dtype=np.float32).astype(bf))
    bq = np.asarray(inputs["bq"], dtype=np.float32)
    bv = np.asarray(inputs["bv"], dtype=np.float32)

    nc = _build()
    bqc = np.ascontiguousarray(np.repeat(bq, HD).reshape(4, 128).T)  # [128, 4]
    peT = _pe_T()                                                    # [D, L]
    xT = np.swapaxes(x, 1, 2)                                        # [B, D, L]
    qkT = (xT + peT[None]).astype(bf)                                # [B, D, L]
    xT = xT.astype(bf)
    base = {"wq": wq, "wk": wk, "wv": wv, "bqc": bqc}
    in_maps = [
        {**base, "xt": np.ascontiguousarray(xT[b]),
         "qkt": np.ascontiguousarray(qkT[b])}
        for b in range(B)
    ]
    res = run_bass_kernel_spmd(
        nc, in_maps, core_ids=list(range(B)), trace=TRACE
    )
    LAST_RESULT = res
    out = np.stack([res.results[b]["out"] for b in range(B)]).astype(np.float32)
    out += np.repeat(bv, HD)[None, None, :]
    return out
